# revision 24
# baseline (speedup 1.0000x reference)
"""Trainium2 Bass kernel: 1024-point FFT of real rows -> (real, imag).

Math: out = FFT_1024(x[b, :]) per row. Two folding levels over the real
input x (U[n] = x[n]+x[1024-n], V[n] = x[n]-x[1024-n]), then a radix-2
split of the half-spectrum k in [1,513) by parity:
  Xr[2m]   = sum_{n<256} Aue[n] cos(2pi n m/512)   + U[256](-1)^m
  Xi[2m]   = sum_{n<256} Avo[n] (-sin(2pi n m/512))
  Xr[2m+1] = sum_{n<256} Auo[n] cos(pi n(2m+1)/512)
  Xi[2m+1] = sum_{n<256} Ave[n] (-sin(pi n(2m+1)/512)) - V[256](-1)^m
with Aue/Auo/Avo/Ave the second-level even/odd folds of U and V. The
remaining half follows from X[1024-k] = conj(X[k]); k=0 is a row sum.

The device computes the four quadrants in TRANSPOSED orientation (freq
on PSUM partitions, batch on the free dim): per 512-row group and
128-wide k-tile, K=256 fp32r matmuls (2 accumulating chunks) with the
quarter-size cos/sin matrices as the stationary operand — 18 matmuls
per group. Every rank-1 edge term rides a dead coefficient row: row 0
of Aue/Auo absorbs U[0] +/- x[512] (all-ones coefficient rows), row 0
of Avo carries U[256] (killed by Cei's zero row, applied to the
even-real quadrant by one extra matmul against the one-hot `alt` row),
and row 0 of Ave carries V[256] via Coi's overridden row 0.

The host ships the four folded arrays (same total bytes as x,
group-blocked so every DMA moves one long contiguous run per partition)
and performs the pure data-expansion assembly: parity interleave,
conjugate mirror, k=0 column, final layout transpose. Inputs ride the
sync queue, real/imag outputs the gpsimd/scalar queues. Pure
data-parallel across 8 cores, no collectives.
"""

import os
import numpy as np

N_FFT = 1024
BATCH = 16384
N_CORES = 8
B_CORE = BATCH // N_CORES  # 2048
P = 128
HALF = 512
QU = 256
GC = 512                   # batch rows per group

_BUILD_CACHE = {}


def _constants():
    n = np.arange(QU, dtype=np.float64)[:, None]
    c = np.arange(QU, dtype=np.float64)[None, :]
    cer = np.cos(2 * np.pi * n * (c + 1) / 512)       # row 0 = 1 (absorbs U0+x512)
    cei = -np.sin(2 * np.pi * n * (c + 1) / 512)      # row 0 = 0 (kills U[256] slot)
    cor = np.cos(np.pi * n * (2 * c + 1) / 512)       # row 0 = 1 (absorbs U0-x512)
    coi = -np.sin(np.pi * n * (2 * c + 1) / 512)
    coi[0, :] = np.where(np.arange(QU) % 2 == 0, -1.0, 1.0)  # carries -V[256](-1)^m
    # one-hot row: (-1)^(p+1) pattern, used by warmup + the ER rank-1 matmul
    alt = np.zeros((P, HALF), dtype=np.float32)
    alt[0, 0::2] = -1.0
    alt[0, 1::2] = 1.0
    f32c = lambda a: np.ascontiguousarray(a.astype(np.float32))
    return f32c(cer), f32c(cei), f32c(cor), f32c(coi), alt


def build_nc(b_core=B_CORE):
    """Build + compile the per-core Bass program (same NEFF on all cores)."""
    import concourse.mybir as mybir
    import concourse.tile as tile
    from concourse import bacc

    f32 = mybir.dt.float32
    f32r = mybir.dt.float32r

    gc = min(GC, b_core)
    n_groups = b_core // gc
    n_kt = QU // P             # 128-wide k-tiles per quadrant (2)

    nc = bacc.Bacc(
        "TRN2", target_bir_lowering=False, debug=False, num_devices=N_CORES
    )

    # the four folded arrays ride one tensor: one DMA per group
    dat_in = nc.dram_tensor(
        "dat", [n_groups, 4, QU, gc], f32r, kind="ExternalInput"
    )
    ARR = {"aue": 0, "avo": 1, "auo": 2, "ave": 3}
    coef_in = {
        name: nc.dram_tensor(name, [QU, QU], f32r, kind="ExternalInput")
        for name in ("cer", "cei", "cor", "coi")
    }
    alt_in = nc.dram_tensor("alt", [P, HALF], f32r, kind="ExternalInput")
    # transposed halves, group-blocked: row r = 4p + slot;
    # slots 0,1 = even-k tiles, 2,3 = odd-k tiles (host interleaves)
    o_rt = nc.dram_tensor("o_rt", [n_groups, HALF, gc], f32, kind="ExternalOutput")
    o_it = nc.dram_tensor("o_it", [n_groups, HALF, gc], f32, kind="ExternalOutput")

    # chunk j / partition p hold row n = 2p+j of the data (and coeff rows)
    dat_r = dat_in.ap().rearrange("g a (p j) b -> g p a j b", j=2)
    coef_r = {k: v.ap().rearrange("(p j) k -> p j k", j=2)
              for k, v in coef_in.items()}
    ort_r = o_rt.ap().rearrange("g (p t) b -> g p t b", t=4)
    oit_r = o_it.ap().rearrange("g (p t) b -> g p t b", t=4)

    with tile.TileContext(nc) as tc:
        with (
            tc.tile_pool(name="const", bufs=1) as cpool,
            tc.tile_pool(name="work", bufs=2) as wpool,
            tc.tile_pool(name="outp", bufs=2) as opool,
            tc.tile_pool(name="psm", bufs=4, space="PSUM") as psm,
        ):
            alt_sb = cpool.tile([P, HALF], f32r)
            nc.sync.dma_start(out=alt_sb, in_=alt_in.ap())

            # HAM warmup: keep the PE busy on `alt` while inputs stream in
            # (borrows a "pr" psum slot; it is released untouched)
            wu = psm.tile([P, gc], f32, tag="pr")
            for w in range(12):
                nc.tensor.matmul(
                    wu[:], lhsT=alt_sb[:, 0:P], rhs=alt_sb[:, 0:gc],
                    start=(w == 0), stop=(w == 11),
                )

            coef_sb = {k: cpool.tile([P, 2, QU], f32r, name=f"coef_{k}") for k in coef_r}
            dat0 = wpool.tile([P, 4, 2, gc], f32r, tag="dat", name="dat0")
            # loads ordered so the first (even-real) matmuls start early
            order = ["cer", "aue", "avo", "cei", "cor", "auo", "coi", "ave"]
            for name in order:
                if name in coef_r:
                    for j in range(2):
                        nc.sync.dma_start(
                            out=coef_sb[name][:, j], in_=coef_r[name][:, j]
                        )
                else:
                    nc.sync.dma_start(
                        out=dat0[:, ARR[name]], in_=dat_r[0][:, ARR[name]]
                    )

            for g in range(n_groups):
                if g == 0:
                    dat_t = dat0
                else:
                    dat_t = wpool.tile([P, 4, 2, gc], f32r, tag="dat",
                                       name=f"dat_{g}")
                    nc.sync.dma_start(out=dat_t[:], in_=dat_r[g])
                dat = {k: dat_t[:, a] for k, a in ARR.items()}

                ortg = opool.tile([P, 4, gc], f32, tag="ortg")
                oitg = opool.tile([P, 4, gc], f32, tag="oitg")

                for kt in range(n_kt):
                    ksl = slice(kt * P, (kt + 1) * P)
                    # even-real: Aue @ Cer + U[256]*(-1)^m (via one-hot alt)
                    per = psm.tile([P, gc], f32, tag="pr")
                    for j in range(2):
                        nc.tensor.matmul(
                            per[:], lhsT=coef_sb["cer"][:, j, ksl],
                            rhs=dat["aue"][:, j], start=(j == 0), stop=False,
                        )
                    nc.tensor.matmul(
                        per[:], lhsT=alt_sb[:, 0:P], rhs=dat["avo"][:, 0],
                        start=False, stop=True,
                    )
                    # even-imag: Avo @ Cei (row 0 of Cei kills the U[256] slot)
                    pei = psm.tile([P, gc], f32, tag="pi")
                    for j in range(2):
                        nc.tensor.matmul(
                            pei[:], lhsT=coef_sb["cei"][:, j, ksl],
                            rhs=dat["avo"][:, j], start=(j == 0), stop=(j == 1),
                        )
                    # odd-real: Auo @ Cor (row 0 = ones absorbs U0 - x512)
                    por = psm.tile([P, gc], f32, tag="pr")
                    for j in range(2):
                        nc.tensor.matmul(
                            por[:], lhsT=coef_sb["cor"][:, j, ksl],
                            rhs=dat["auo"][:, j], start=(j == 0), stop=(j == 1),
                        )
                    # odd-imag: Ave @ Coi (row 0 overridden, carries V[256])
                    poi = psm.tile([P, gc], f32, tag="pi")
                    for j in range(2):
                        nc.tensor.matmul(
                            poi[:], lhsT=coef_sb["coi"][:, j, ksl],
                            rhs=dat["ave"][:, j], start=(j == 0), stop=(j == 1),
                        )

                    nc.vector.tensor_copy(out=ortg[:, kt], in_=per[:])
                    nc.vector.tensor_copy(out=ortg[:, 2 + kt], in_=por[:])
                    nc.scalar.copy(out=oitg[:, kt], in_=pei[:])
                    nc.scalar.copy(out=oitg[:, 2 + kt], in_=poi[:])

                # group-blocked outputs: one 8KB run per partition
                nc.gpsimd.dma_start(out=ort_r[g], in_=ortg[:])
                nc.scalar.dma_start(out=oit_r[g], in_=oitg[:])

    nc.compile()
    return nc


def _get_nc(b_core=B_CORE):
    if b_core not in _BUILD_CACHE:
        _BUILD_CACHE[b_core] = build_nc(b_core)
    return _BUILD_CACHE[b_core]


def _host_prep(x):
    """Two-level real-FFT folds (transposed) + host-side k=0 column."""
    B = x.shape[0]
    U = np.empty((B, HALF), dtype=np.float32)
    V = np.empty((B, HALF), dtype=np.float32)
    U[:, 0] = x[:, 0]
    rev = x[:, 1023:HALF:-1]
    np.add(x[:, 1:HALF], rev, out=U[:, 1:HALF])
    np.subtract(x[:, 1:HALF], rev, out=V[:, 1:HALF])
    x512 = x[:, HALF]
    a = {k: np.empty((B, QU), dtype=np.float32)
         for k in ("aue", "auo", "avo", "ave")}
    a["aue"][:, 0] = U[:, 0] + x512
    a["auo"][:, 0] = U[:, 0] - x512
    a["avo"][:, 0] = U[:, QU]                  # = x[256] + x[768]
    a["ave"][:, 0] = V[:, QU]                  # = x[256] - x[768]
    urev = U[:, 511:QU:-1]
    vrev = V[:, 511:QU:-1]
    np.add(U[:, 1:QU], urev, out=a["aue"][:, 1:QU])
    np.subtract(U[:, 1:QU], urev, out=a["auo"][:, 1:QU])
    np.subtract(V[:, 1:QU], vrev, out=a["avo"][:, 1:QU])
    np.add(V[:, 1:QU], vrev, out=a["ave"][:, 1:QU])
    col0 = (U.sum(axis=1, dtype=np.float64) + x512).astype(np.float32)
    at = {k: np.ascontiguousarray(v.T) for k, v in a.items()}   # [256, B]
    return at, col0


def _blocked(a_t, sl, b_core):
    """[256, B] column-slice -> group-blocked [n_groups, 256, gc] contiguous."""
    gc = min(GC, b_core)
    n_groups = b_core // gc
    s = a_t[:, sl]
    return np.ascontiguousarray(s.reshape(QU, n_groups, gc).transpose(1, 0, 2))


def _assemble(half_t, out, sl, b_core, neg_mirror):
    """Device half [n_groups, 512(r=4p+slot), gc] -> out[sl, :] (1024 cols).

    slot 0,1: even k = 2*(kt*128 + p + 1); slot 2,3: odd k = 2*(kt*128+p)+1.
    """
    gc = min(GC, b_core)
    n_groups = b_core // gc
    h = half_t.reshape(n_groups, P, 4, gc)
    b0 = sl.start
    for g in range(n_groups):
        rows = slice(b0 + g * gc, b0 + (g + 1) * gc)
        for kt in range(2):
            e0 = 2 * (kt * P) + 2
            out[rows, e0 : e0 + 2 * P : 2] = h[g, :, kt, :].T
            o0 = 2 * (kt * P) + 1
            out[rows, o0 : o0 + 2 * P : 2] = h[g, :, 2 + kt, :].T
    blk = out[sl]
    if neg_mirror:
        np.negative(blk[:, 511:0:-1], out=blk[:, 513:1024])
    else:
        blk[:, 513:1024] = blk[:, 511:0:-1]


def kernel(**inputs):
    from concourse.bass_utils import run_bass_kernel_spmd

    x = np.ascontiguousarray(np.asarray(inputs["x"], dtype=np.float32))
    assert x.shape == (BATCH, N_FFT), x.shape
    cer, cei, cor, coi, alt = _constants()
    at, col0 = _host_prep(x)
    nc = _get_nc()
    in_maps = []
    for c in range(N_CORES):
        sl = slice(c * B_CORE, (c + 1) * B_CORE)
        blocks = [_blocked(at[k], sl, B_CORE)
                  for k in ("aue", "avo", "auo", "ave")]
        m = {"dat": np.ascontiguousarray(np.stack(blocks, axis=1))}
        m.update({"cer": cer, "cei": cei, "cor": cor, "coi": coi, "alt": alt})
        in_maps.append(m)
    trace = bool(int(os.environ.get("FFT_KERNEL_TRACE", "0")))
    try:
        res = run_bass_kernel_spmd(
            nc, in_maps, core_ids=list(range(N_CORES)), trace=trace
        )
    except Exception:
        # transient NRT/device hiccups have been observed; retry once
        res = run_bass_kernel_spmd(
            nc, in_maps, core_ids=list(range(N_CORES)), trace=trace
        )
    if trace:
        kernel.last_results = res
    real = np.empty((BATCH, N_FFT), dtype=np.float32)
    imag = np.empty((BATCH, N_FFT), dtype=np.float32)
    for c in range(N_CORES):
        sl = slice(c * B_CORE, (c + 1) * B_CORE)
        _assemble(res.results[c]["o_rt"], real, sl, B_CORE, neg_mirror=False)
        _assemble(res.results[c]["o_it"], imag, sl, B_CORE, neg_mirror=True)
    real[:, 0] = col0
    imag[:, 0] = 0.0
    return real, imag


# revision 25
# speedup vs baseline: 1.0801x; 1.0801x over previous
"""Trainium2 Bass kernel: 1024-point FFT of real rows -> (real, imag).

Math: out = FFT_1024(x[b, :]) per row. Two folding levels over the real
input x (U[n] = x[n]+x[1024-n], V[n] = x[n]-x[1024-n]), then a radix-2
split of the half-spectrum k in [1,513) by parity:
  Xr[2m]   = sum_{n<256} Aue[n] cos(2pi n m/512)   + U[256](-1)^m
  Xi[2m]   = sum_{n<256} Avo[n] (-sin(2pi n m/512))
  Xr[2m+1] = sum_{n<256} Auo[n] cos(pi n(2m+1)/512)
  Xi[2m+1] = sum_{n<256} Ave[n] (-sin(pi n(2m+1)/512)) - V[256](-1)^m
with Aue/Auo/Avo/Ave the second-level even/odd folds of U and V. The
remaining half follows from X[1024-k] = conj(X[k]); k=0 is a row sum.

The device computes the four quadrants in TRANSPOSED orientation (freq
on PSUM partitions, batch on the free dim): per 512-row group and
128-wide k-tile, K=256 fp32r matmuls (2 accumulating chunks) with the
quarter-size cos/sin matrices as the stationary operand — 18 matmuls
per group. Every rank-1 edge term rides a dead coefficient row: row 0
of Aue/Auo absorbs U[0] +/- x[512] (all-ones coefficient rows), row 0
of Avo carries U[256] (killed by Cei's zero row, applied to the
even-real quadrant by one extra matmul against the one-hot `alt` row),
and row 0 of Ave carries V[256] via Coi's overridden row 0.

The host ships the four folded arrays (same total bytes as x,
group-blocked so every DMA moves one long contiguous run per partition)
and performs the pure data-expansion assembly: parity interleave,
conjugate mirror, k=0 column, final layout transpose. Inputs ride the
sync queue, real/imag outputs the gpsimd/scalar queues. Pure
data-parallel across 8 cores, no collectives.
"""

import os
import numpy as np

N_FFT = 1024
BATCH = 16384
N_CORES = 8
B_CORE = BATCH // N_CORES  # 2048
P = 128
HALF = 512
QU = 256
GC = 512                   # batch rows per group

_BUILD_CACHE = {}


def _constants():
    n = np.arange(QU, dtype=np.float64)[:, None]
    c = np.arange(QU, dtype=np.float64)[None, :]
    cer = np.cos(2 * np.pi * n * (c + 1) / 512)       # row 0 = 1 (absorbs U0+x512)
    cei = -np.sin(2 * np.pi * n * (c + 1) / 512)      # row 0 = 0 (kills U[256] slot)
    cor = np.cos(np.pi * n * (2 * c + 1) / 512)       # row 0 = 1 (absorbs U0-x512)
    coi = -np.sin(np.pi * n * (2 * c + 1) / 512)
    coi[0, :] = np.where(np.arange(QU) % 2 == 0, -1.0, 1.0)  # carries -V[256](-1)^m
    # one-hot row: (-1)^(p+1) pattern, used by warmup + the ER rank-1 matmul
    alt = np.zeros((P, HALF), dtype=np.float32)
    alt[0, 0::2] = -1.0
    alt[0, 1::2] = 1.0
    f32c = lambda a: np.ascontiguousarray(a.astype(np.float32))
    return f32c(cer), f32c(cei), f32c(cor), f32c(coi), alt


def build_nc(b_core=B_CORE):
    """Build + compile the per-core Bass program (same NEFF on all cores)."""
    import concourse.mybir as mybir
    import concourse.tile as tile
    from concourse import bacc

    f32 = mybir.dt.float32
    f32r = mybir.dt.float32r

    gc = min(GC, b_core)
    n_groups = b_core // gc
    n_kt = QU // P             # 128-wide k-tiles per quadrant (2)

    nc = bacc.Bacc(
        "TRN2", target_bir_lowering=False, debug=False, num_devices=N_CORES
    )

    # the four folded arrays ride one tensor: one DMA per group
    dat_in = nc.dram_tensor(
        "dat", [n_groups, 4, QU, gc], f32r, kind="ExternalInput"
    )
    ARR = {"aue": 0, "avo": 1, "auo": 2, "ave": 3}
    coef_in = {
        name: nc.dram_tensor(name, [QU, QU], f32r, kind="ExternalInput")
        for name in ("cer", "cei", "cor", "coi")
    }
    alt_in = nc.dram_tensor("alt", [P, HALF], f32r, kind="ExternalInput")
    # transposed halves, group-blocked: row r = 4p + slot;
    # slots 0,1 = even-k tiles, 2,3 = odd-k tiles (host interleaves)
    o_rt = nc.dram_tensor("o_rt", [n_groups, HALF, gc], f32, kind="ExternalOutput")
    o_it = nc.dram_tensor("o_it", [n_groups, HALF, gc], f32, kind="ExternalOutput")

    # chunk j / partition p hold row n = 2p+j of the data (and coeff rows)
    dat_r = dat_in.ap().rearrange("g a (p j) b -> g p a j b", j=2)
    coef_r = {k: v.ap().rearrange("(p j) k -> p j k", j=2)
              for k, v in coef_in.items()}
    ort_r = o_rt.ap().rearrange("g (p t) b -> g p t b", t=4)
    oit_r = o_it.ap().rearrange("g (p t) b -> g p t b", t=4)

    with tile.TileContext(nc) as tc:
        with (
            tc.tile_pool(name="const", bufs=1) as cpool,
            tc.tile_pool(name="work", bufs=2) as wpool,
            tc.tile_pool(name="outp", bufs=2) as opool,
            tc.tile_pool(name="psm", bufs=4, space="PSUM") as psm,
        ):
            alt_sb = cpool.tile([P, HALF], f32r)
            nc.sync.dma_start(out=alt_sb, in_=alt_in.ap())

            # HAM warmup: keep the PE busy on `alt` while inputs stream in
            # (borrows a "pr" psum slot; it is released untouched)
            wu = psm.tile([P, gc], f32, tag="pr")
            for w in range(12):
                nc.tensor.matmul(
                    wu[:], lhsT=alt_sb[:, 0:P], rhs=alt_sb[:, 0:gc],
                    start=(w == 0), stop=(w == 11),
                )

            coef_sb = {k: cpool.tile([P, 2, QU], f32r, name=f"coef_{k}") for k in coef_r}
            dat0 = wpool.tile([P, 4, 2, gc], f32r, tag="dat", name="dat0")
            # loads ordered so the first (even-real) matmuls start early
            order = ["cer", "aue", "avo", "cei", "cor", "auo", "coi", "ave"]
            for name in order:
                if name in coef_r:
                    for j in range(2):
                        nc.sync.dma_start(
                            out=coef_sb[name][:, j], in_=coef_r[name][:, j]
                        )
                else:
                    nc.sync.dma_start(
                        out=dat0[:, ARR[name]], in_=dat_r[0][:, ARR[name]]
                    )

            for g in range(n_groups):
                if g == 0:
                    dat_t = dat0
                else:
                    dat_t = wpool.tile([P, 4, 2, gc], f32r, tag="dat",
                                       name=f"dat_{g}")
                    nc.sync.dma_start(out=dat_t[:, 0:2], in_=dat_r[g][:, 0:2])
                    nc.sync.dma_start(out=dat_t[:, 2:4], in_=dat_r[g][:, 2:4])
                dat = {k: dat_t[:, a] for k, a in ARR.items()}

                ortg = opool.tile([P, 4, gc], f32, tag="ortg")
                oitg = opool.tile([P, 4, gc], f32, tag="oitg")

                for kt in range(n_kt):
                    ksl = slice(kt * P, (kt + 1) * P)
                    # even-real: Aue @ Cer + U[256]*(-1)^m (via one-hot alt)
                    per = psm.tile([P, gc], f32, tag="pr")
                    for j in range(2):
                        nc.tensor.matmul(
                            per[:], lhsT=coef_sb["cer"][:, j, ksl],
                            rhs=dat["aue"][:, j], start=(j == 0), stop=False,
                        )
                    nc.tensor.matmul(
                        per[:], lhsT=alt_sb[:, 0:P], rhs=dat["avo"][:, 0],
                        start=False, stop=True,
                    )
                    # even-imag: Avo @ Cei (row 0 of Cei kills the U[256] slot)
                    pei = psm.tile([P, gc], f32, tag="pi")
                    for j in range(2):
                        nc.tensor.matmul(
                            pei[:], lhsT=coef_sb["cei"][:, j, ksl],
                            rhs=dat["avo"][:, j], start=(j == 0), stop=(j == 1),
                        )
                    # odd-real: Auo @ Cor (row 0 = ones absorbs U0 - x512)
                    por = psm.tile([P, gc], f32, tag="pr")
                    for j in range(2):
                        nc.tensor.matmul(
                            por[:], lhsT=coef_sb["cor"][:, j, ksl],
                            rhs=dat["auo"][:, j], start=(j == 0), stop=(j == 1),
                        )
                    # odd-imag: Ave @ Coi (row 0 overridden, carries V[256])
                    poi = psm.tile([P, gc], f32, tag="pi")
                    for j in range(2):
                        nc.tensor.matmul(
                            poi[:], lhsT=coef_sb["coi"][:, j, ksl],
                            rhs=dat["ave"][:, j], start=(j == 0), stop=(j == 1),
                        )

                    nc.vector.tensor_copy(out=ortg[:, kt], in_=per[:])
                    nc.vector.tensor_copy(out=ortg[:, 2 + kt], in_=por[:])
                    nc.scalar.copy(out=oitg[:, kt], in_=pei[:])
                    nc.scalar.copy(out=oitg[:, 2 + kt], in_=poi[:])

                # group-blocked outputs: one 8KB run per partition
                nc.gpsimd.dma_start(out=ort_r[g], in_=ortg[:])
                nc.scalar.dma_start(out=oit_r[g], in_=oitg[:])

    nc.compile()
    return nc


def _get_nc(b_core=B_CORE):
    if b_core not in _BUILD_CACHE:
        _BUILD_CACHE[b_core] = build_nc(b_core)
    return _BUILD_CACHE[b_core]


def _host_prep(x):
    """Two-level real-FFT folds (transposed) + host-side k=0 column."""
    B = x.shape[0]
    U = np.empty((B, HALF), dtype=np.float32)
    V = np.empty((B, HALF), dtype=np.float32)
    U[:, 0] = x[:, 0]
    rev = x[:, 1023:HALF:-1]
    np.add(x[:, 1:HALF], rev, out=U[:, 1:HALF])
    np.subtract(x[:, 1:HALF], rev, out=V[:, 1:HALF])
    x512 = x[:, HALF]
    a = {k: np.empty((B, QU), dtype=np.float32)
         for k in ("aue", "auo", "avo", "ave")}
    a["aue"][:, 0] = U[:, 0] + x512
    a["auo"][:, 0] = U[:, 0] - x512
    a["avo"][:, 0] = U[:, QU]                  # = x[256] + x[768]
    a["ave"][:, 0] = V[:, QU]                  # = x[256] - x[768]
    urev = U[:, 511:QU:-1]
    vrev = V[:, 511:QU:-1]
    np.add(U[:, 1:QU], urev, out=a["aue"][:, 1:QU])
    np.subtract(U[:, 1:QU], urev, out=a["auo"][:, 1:QU])
    np.subtract(V[:, 1:QU], vrev, out=a["avo"][:, 1:QU])
    np.add(V[:, 1:QU], vrev, out=a["ave"][:, 1:QU])
    col0 = (U.sum(axis=1, dtype=np.float64) + x512).astype(np.float32)
    at = {k: np.ascontiguousarray(v.T) for k, v in a.items()}   # [256, B]
    return at, col0


def _blocked(a_t, sl, b_core):
    """[256, B] column-slice -> group-blocked [n_groups, 256, gc] contiguous."""
    gc = min(GC, b_core)
    n_groups = b_core // gc
    s = a_t[:, sl]
    return np.ascontiguousarray(s.reshape(QU, n_groups, gc).transpose(1, 0, 2))


def _assemble(half_t, out, sl, b_core, neg_mirror):
    """Device half [n_groups, 512(r=4p+slot), gc] -> out[sl, :] (1024 cols).

    slot 0,1: even k = 2*(kt*128 + p + 1); slot 2,3: odd k = 2*(kt*128+p)+1.
    """
    gc = min(GC, b_core)
    n_groups = b_core // gc
    h = half_t.reshape(n_groups, P, 4, gc)
    b0 = sl.start
    for g in range(n_groups):
        rows = slice(b0 + g * gc, b0 + (g + 1) * gc)
        for kt in range(2):
            e0 = 2 * (kt * P) + 2
            out[rows, e0 : e0 + 2 * P : 2] = h[g, :, kt, :].T
            o0 = 2 * (kt * P) + 1
            out[rows, o0 : o0 + 2 * P : 2] = h[g, :, 2 + kt, :].T
    blk = out[sl]
    if neg_mirror:
        np.negative(blk[:, 511:0:-1], out=blk[:, 513:1024])
    else:
        blk[:, 513:1024] = blk[:, 511:0:-1]


def kernel(**inputs):
    from concourse.bass_utils import run_bass_kernel_spmd

    x = np.ascontiguousarray(np.asarray(inputs["x"], dtype=np.float32))
    assert x.shape == (BATCH, N_FFT), x.shape
    cer, cei, cor, coi, alt = _constants()
    at, col0 = _host_prep(x)
    nc = _get_nc()
    in_maps = []
    for c in range(N_CORES):
        sl = slice(c * B_CORE, (c + 1) * B_CORE)
        blocks = [_blocked(at[k], sl, B_CORE)
                  for k in ("aue", "avo", "auo", "ave")]
        m = {"dat": np.ascontiguousarray(np.stack(blocks, axis=1))}
        m.update({"cer": cer, "cei": cei, "cor": cor, "coi": coi, "alt": alt})
        in_maps.append(m)
    trace = bool(int(os.environ.get("FFT_KERNEL_TRACE", "0")))
    try:
        res = run_bass_kernel_spmd(
            nc, in_maps, core_ids=list(range(N_CORES)), trace=trace
        )
    except Exception:
        # transient NRT/device hiccups have been observed; retry once
        res = run_bass_kernel_spmd(
            nc, in_maps, core_ids=list(range(N_CORES)), trace=trace
        )
    if trace:
        kernel.last_results = res
    real = np.empty((BATCH, N_FFT), dtype=np.float32)
    imag = np.empty((BATCH, N_FFT), dtype=np.float32)
    for c in range(N_CORES):
        sl = slice(c * B_CORE, (c + 1) * B_CORE)
        _assemble(res.results[c]["o_rt"], real, sl, B_CORE, neg_mirror=False)
        _assemble(res.results[c]["o_it"], imag, sl, B_CORE, neg_mirror=True)
    real[:, 0] = col0
    imag[:, 0] = 0.0
    return real, imag
